# revision 1
# baseline (speedup 1.0000x reference)
"""Trainium2 Bass kernel for ContextQueryAttn (BiDAF-style trilinear attention).

Computes, per batch b:
    sim = sc[:,None] + sq[None,:] + (ctx*wm) @ query.T          (Lc, Lq)
    sim = where(cmask[:,None] | qmask[None,:], -1e30, sim)
    S   = softmax(sim, axis=-1)   (row softmax over Lq)
    SS  = softmax(sim, axis=0)    (col softmax over Lc)
    A   = S @ query               (Lc, D)
    T   = SS.T @ ctx              (Lq, D)
    B   = S @ T                   (Lc, D)
returns (A, B).

Strategy: data-parallel over batch B=32 across 8 cores (4 batches/core).
All matmuls on PE in float32r (fp22 mantissa, full speed at N>=256).
Softmaxes use no max-subtraction (logits are O(+-10); masked entries hit
exp(-1e30)=0 exactly); normalizers come from ones-columns appended to the
matmul RHS operands; fully-masked rows/cols reproduce the reference's
uniform-softmax semantics via ACT scale/bias folding and a predicated copy.
"""

import os
import numpy as np

import concourse.bass as bass
import concourse.tile as tile
from concourse import bacc, mybir
from concourse.bass_utils import run_bass_kernel_spmd

F32 = mybir.dt.float32
F32R = mybir.dt.float32r
EXP = mybir.ActivationFunctionType.Exp
ALU = mybir.AluOpType

B, LC, LQ, D = 32, 2048, 256, 256
NCORES = 8
BPC = B // NCORES          # batches per core
NCT = LC // 128            # 16 context tiles
NQT = LQ // 128            # 2 query tiles
NKD = D // 128             # 2 contraction chunks over D
NCH = LC // 512            # 4 dotT column chunks
NEG = np.float32(-1e30)

# Transposes in plain f32 (2 cyc/row) by default; f32r (1.5) is an option.
TRANSPOSE_DT = F32R


def _build_kernel(tc, nc, ins, outs):
    import contextlib
    ctx = contextlib.ExitStack()

    sb = lambda name, bufs: ctx.enter_context(
        tc.tile_pool(name=name, bufs=bufs))
    ps_pool = ctx.enter_context(tc.tile_pool(name="ps", bufs=6, space="PSUM"))
    t_pool = ctx.enter_context(tc.tile_pool(name="tps", bufs=1, space="PSUM"))

    p_const = sb("const", 1)
    p_ctx = sb("pctx", 2)
    p_ctxT = sb("pctxT", 2)
    p_PT = sb("pPT", 2)
    p_Pc = sb("pPc", 2)
    p_q = sb("pq", 2)
    p_qwmT = sb("pqwmT", 2)
    p_Tn = sb("pTn", 2)
    p_cm = sb("pcm", 2)
    p_cs = sb("pcs", 2)
    p_vec = sb("pvec", 2)
    p_stage = sb("pstage", 8)

    ident = p_const.tile([128, 128], F32R, name="ident")
    nc.sync.dma_start(out=ident[:], in_=ins["ident"])

    r128 = lambda ap: ap.rearrange("(t p) x -> p t x", p=128)
    v128 = lambda ap: ap.rearrange("(t p) -> p t", p=128)

    for b in range(BPC):
        # ---- loads ----
        ctx_sb = p_ctx.tile([128, NCT, 258], F32R, name="ctx_sb")
        nc.sync.dma_start(out=ctx_sb[:], in_=r128(ins["ctx_ext"][b]))
        q_sb = p_q.tile([128, NQT, 258], F32R, name="q_sb")
        nc.sync.dma_start(out=q_sb[:], in_=r128(ins["query_ext"][b]))
        qwmT_sb = p_qwmT.tile([128, NKD, LQ], F32R, name="qwmT_sb")
        nc.sync.dma_start(out=qwmT_sb[:], in_=r128(ins["qwmT"][b]))

        sqb_sb = p_vec.tile([128, NQT], F32, name="sqb_sb")
        nc.sync.dma_start(out=sqb_sb[:], in_=v128(ins["sq_bias"][b]))
        nbs_sb = p_vec.tile([128, NQT], F32, name="nbs_sb")
        nc.sync.dma_start(out=nbs_sb[:], in_=v128(ins["nbs"][b]))
        qsc_sb = p_vec.tile([128, NQT], F32, name="qsc_sb")
        nc.sync.dma_start(out=qsc_sb[:], in_=v128(ins["q_scale"][b]))
        qf_sb = p_vec.tile([128, NQT], F32, name="qf_sb")
        nc.sync.dma_start(out=qf_sb[:], in_=v128(ins["qf"][b]))
        scb_sb = p_vec.tile([128, NCT], F32, name="scb_sb")
        nc.sync.dma_start(out=scb_sb[:], in_=v128(ins["sc_bias"][b]))
        csc_sb = p_vec.tile([128, NCT], F32, name="csc_sb")
        nc.sync.dma_start(out=csc_sb[:], in_=v128(ins["c_scale"][b]))

        cmaskb_sb = p_cm.tile([128, LC], mybir.dt.uint8, name="cmaskb_sb")
        nc.sync.dma_start(out=cmaskb_sb[:],
                          in_=ins["cmask"][b][None, :].to_broadcast((128, LC)))
        ctxsum_sb = p_cs.tile([128, 258], F32, name="ctxsum_sb")
        nc.sync.dma_start(out=ctxsum_sb[:],
                          in_=ins["ctxsum_ext"][b][None, :].to_broadcast((128, 258)))

        rsrec_sb = p_vec.tile([128, NCT], F32, name="rsrec_sb")
        csrec_sb = p_vec.tile([128, NQT], F32, name="csrec_sb")

        # ---- ctx transposes: ctxT[kd] = ctx[:, kd-block].T  (d on partitions)
        ctxT_sb = p_ctxT.tile([128, NKD, LC], F32R, name="ctxT_sb")
        for kd in range(NKD):
            for g in range(NCH):
                tp = ps_pool.tile([128, 512], F32R, tag="ps", name="tp")
                for j in range(4):
                    ci = 4 * g + j
                    nc.tensor.transpose(
                        out=tp[:, bass.ts(j, 128)],
                        in_=ctx_sb[:, ci, bass.ts(kd, 128)],
                        identity=ident[:])
                nc.scalar.copy(ctxT_sb[:, kd, bass.ts(g, 512)], tp[:])

        # ---- row path: dotT (q, c) -> exp -> P^T, cmask predicated copy
        PT_sb = p_PT.tile([128, NQT, LC], F32R, name="PT_sb")
        for qt in range(NQT):
            for ch in range(NCH):
                dt_ps = ps_pool.tile([128, 512], F32, tag="ps", name="dt_ps")
                for kd in range(NKD):
                    nc.tensor.matmul(
                        dt_ps[:],
                        lhsT=qwmT_sb[:, kd, bass.ts(qt, 128)],
                        rhs=ctxT_sb[:, kd, bass.ts(ch, 512)],
                        start=(kd == 0), stop=(kd == NKD - 1))
                # cmasked columns -> -sq_bias[q], cancelling the exp bias
                # exactly: exp(0)=1 (uniform row), incl. qmasked rows where
                # +1e30 + (-1e30) = 0.
                nc.vector.copy_predicated(
                    out=dt_ps[:], mask=cmaskb_sb[:, bass.ts(ch, 512)],
                    data=nbs_sb[:, qt:qt + 1].to_broadcast((128, 512)))
                nc.scalar.activation(
                    PT_sb[:, qt, bass.ts(ch, 512)], dt_ps[:], EXP,
                    bias=sqb_sb[:, qt:qt + 1])

        # ---- col path: dot (c, q) -> exp -> Pc; T accumulation
        T_ps = [t_pool.tile([128, 258], F32, name=f"T_ps{qt}") for qt in range(NQT)]
        Pc_sb = p_Pc.tile([128, NCT, LQ], F32R, name="Pc_sb")
        for ci in range(NCT):
            dps = ps_pool.tile([128, LQ], F32, tag="ps", name="dps")
            for kd in range(NKD):
                nc.tensor.matmul(
                    dps[:],
                    lhsT=ctxT_sb[:, kd, bass.ts(ci, 128)],
                    rhs=qwmT_sb[:, kd, :],
                    start=(kd == 0), stop=(kd == NKD - 1))
            nc.scalar.activation(
                Pc_sb[:, ci, :], dps[:], EXP,
                bias=scb_sb[:, ci:ci + 1], scale=csc_sb[:, ci:ci + 1])
            for qt in range(NQT):
                nc.tensor.matmul(
                    T_ps[qt][:],
                    lhsT=Pc_sb[:, ci, bass.ts(qt, 128)],
                    rhs=ctx_sb[:, ci, :],
                    start=(ci == 0), stop=(ci == NCT - 1))
            # A path interleaved: independent PE work while ACT runs exps
            a_ps = ps_pool.tile([128, 258], F32, tag="ps", name="a_ps")
            for qt in range(NQT):
                nc.tensor.matmul(
                    a_ps[:],
                    lhsT=PT_sb[:, qt, bass.ts(ci, 128)],
                    rhs=q_sb[:, qt, :],
                    start=(qt == 0), stop=(qt == NQT - 1))
            nc.vector.reciprocal(rsrec_sb[:, ci:ci + 1], a_ps[:, 256:257])
            a_st = p_stage.tile([128, 256], F32, tag="ast", name="a_st")
            nc.scalar.mul(a_st[:], a_ps[:, 0:256], rsrec_sb[:, ci:ci + 1])
            nc.sync.dma_start(out=outs["A"][b, bass.ts(ci, 128), :], in_=a_st[:])

        # ---- T finalize: blend qmask + normalize
        Tn_sb = p_Tn.tile([128, NQT, 256], F32R, name="Tn_sb")
        for qt in range(NQT):
            nc.vector.tensor_scalar_mul(
                T_ps[qt][:], T_ps[qt][:], qsc_sb[:, qt:qt + 1])
            nc.vector.scalar_tensor_tensor(
                out=T_ps[qt][:], in0=ctxsum_sb[:], scalar=qf_sb[:, qt:qt + 1],
                in1=T_ps[qt][:], op0=ALU.mult, op1=ALU.add)
            nc.vector.reciprocal(csrec_sb[:, qt:qt + 1], T_ps[qt][:, 256:257])
            nc.scalar.mul(Tn_sb[:, qt, :], T_ps[qt][:, 0:256],
                          csrec_sb[:, qt:qt + 1])

        # ---- Bmat = S @ T
        for ci in range(NCT):
            b_ps = ps_pool.tile([128, 256], F32, tag="ps", name="b_ps")
            for qt in range(NQT):
                nc.tensor.matmul(
                    b_ps[:],
                    lhsT=PT_sb[:, qt, bass.ts(ci, 128)],
                    rhs=Tn_sb[:, qt, :],
                    start=(qt == 0), stop=(qt == NQT - 1))
            b_st = p_stage.tile([128, 256], F32, tag="bst", name="b_st")
            nc.vector.tensor_scalar_mul(b_st[:], b_ps[:], rsrec_sb[:, ci:ci + 1])
            nc.sync.dma_start(out=outs["Bm"][b, bass.ts(ci, 128), :], in_=b_st[:])

    ctx.close()


def build_program():
    nc = bacc.Bacc("TRN2", target_bir_lowering=False, debug=False,
                   num_devices=NCORES)
    ins = {
        "ctx_ext": nc.dram_tensor("ctx_ext", [BPC, LC, 258], F32R,
                                  kind="ExternalInput").ap(),
        "query_ext": nc.dram_tensor("query_ext", [BPC, LQ, 258], F32R,
                                    kind="ExternalInput").ap(),
        "qwmT": nc.dram_tensor("qwmT", [BPC, D, LQ], F32R,
                               kind="ExternalInput").ap(),
        "sq_bias": nc.dram_tensor("sq_bias", [BPC, LQ], F32,
                                  kind="ExternalInput").ap(),
        "q_scale": nc.dram_tensor("q_scale", [BPC, LQ], F32,
                                  kind="ExternalInput").ap(),
        "qf": nc.dram_tensor("qf", [BPC, LQ], F32, kind="ExternalInput").ap(),
        "sc_bias": nc.dram_tensor("sc_bias", [BPC, LC], F32,
                                  kind="ExternalInput").ap(),
        "c_scale": nc.dram_tensor("c_scale", [BPC, LC], F32,
                                  kind="ExternalInput").ap(),
        "cmask": nc.dram_tensor("cmask", [BPC, LC], mybir.dt.uint8,
                                kind="ExternalInput").ap(),
        "ctxsum_ext": nc.dram_tensor("ctxsum_ext", [BPC, 258], F32,
                                     kind="ExternalInput").ap(),
        "nbs": nc.dram_tensor("nbs", [BPC, LQ], F32,
                              kind="ExternalInput").ap(),
        "ident": nc.dram_tensor("ident", [128, 128], F32R,
                                kind="ExternalInput").ap(),
    }
    outs = {
        "A": nc.dram_tensor("A", [BPC, LC, D], F32, kind="ExternalOutput").ap(),
        "Bm": nc.dram_tensor("Bm", [BPC, LC, D], F32, kind="ExternalOutput").ap(),
    }
    with tile.TileContext(nc) as tc:
        _build_kernel(tc, nc, ins, outs)
    nc.compile()
    return nc


def host_prep(context, query, context_mask, query_mask, w0):
    """Host-side preprocessing: shard + build auxiliary tensors (all O(B*L*D))."""
    f = np.float32
    context = np.ascontiguousarray(context, dtype=f)
    query = np.ascontiguousarray(query, dtype=f)
    w0 = np.asarray(w0, dtype=f)
    wc, wq, wm = w0[:D], w0[D:2 * D], w0[2 * D:]
    cf = context_mask.astype(f)
    qf = query_mask.astype(f)
    sc = context @ wc                      # (B, LC)
    sq = query @ wq                        # (B, LQ)
    qwmT = np.ascontiguousarray((query * wm).transpose(0, 2, 1))
    ones_c = np.ones((B, LC, 1), f)
    ones_q = np.ones((B, LQ, 1), f)
    zc = np.zeros((B, LC, 1), f)
    zq = np.zeros((B, LQ, 1), f)
    ctx_ext = np.ascontiguousarray(np.concatenate([context, ones_c, zc], -1))
    query_ext = np.ascontiguousarray(np.concatenate([query, ones_q, zq], -1))
    ctxsum_ext = np.concatenate(
        [context.sum(1, dtype=f), np.full((B, 1), LC, f),
         np.zeros((B, 1), f)], -1)
    q_scale = (1.0 - qf).astype(f)
    sq_bias = (q_scale * sq + qf * NEG).astype(f)
    c_scale = (1.0 - cf).astype(f)
    sc_bias = (c_scale * sc + cf * NEG).astype(f)

    full = {
        "ctx_ext": ctx_ext, "query_ext": query_ext, "qwmT": qwmT,
        "sq_bias": sq_bias, "nbs": -sq_bias, "q_scale": q_scale, "qf": qf,
        "sc_bias": sc_bias, "c_scale": c_scale,
        "cmask": cf.astype(np.uint8),
        "ctxsum_ext": ctxsum_ext,
    }
    const = {"ident": np.eye(128, dtype=f)}
    in_maps = []
    for c in range(NCORES):
        sl = slice(c * BPC, (c + 1) * BPC)
        m = {k: np.ascontiguousarray(v[sl]) for k, v in full.items()}
        m.update(const)
        in_maps.append(m)
    return in_maps


_cached_nc = None


def get_program():
    global _cached_nc
    if _cached_nc is None:
        _cached_nc = build_program()
    return _cached_nc


def run_on_hw(in_maps, **kwargs):
    nc = get_program()
    return run_bass_kernel_spmd(nc, in_maps, core_ids=list(range(NCORES)),
                                **kwargs)


def kernel(context, query, context_mask, query_mask, w0):
    in_maps = host_prep(context, query, context_mask, query_mask, w0)
    res = run_on_hw(in_maps)
    A = np.concatenate([res.results[c]["A"] for c in range(NCORES)], 0)
    Bm = np.concatenate([res.results[c]["Bm"] for c in range(NCORES)], 0)
    return A, Bm



# revision 2
# speedup vs baseline: 2.3668x; 2.3668x over previous
"""Trainium2 Bass kernel for ContextQueryAttn (BiDAF-style trilinear attention).

Computes, per batch b:
    sim = sc[:,None] + sq[None,:] + (ctx*wm) @ query.T          (Lc, Lq)
    sim = where(cmask[:,None] | qmask[None,:], -1e30, sim)
    S   = softmax(sim, axis=-1)   (row softmax over Lq)
    SS  = softmax(sim, axis=0)    (col softmax over Lc)
    A   = S @ query               (Lc, D)
    T   = SS.T @ ctx              (Lq, D)
    B   = S @ T                   (Lc, D)
returns (A, B).

v2 strategy:
 - data-parallel over batch B=32 across 8 cores (4 batches/core)
 - context-mask COMPACTION: masked context rows produce uniform-softmax
   outputs (constant rows) that the host reconstructs exactly; the device
   only processes gathered unmasked rows (~1024 of 2048), padded to a
   multiple of 128 chosen at runtime from the data.
 - all matmul operands bf16 (FWL weight loads, half DMA/SBUF), f32 PSUM.
 - ctx^T precomputed on host (no PE transposes on device).
 - all DRAM tensors stored in SBUF-image layout [*, 128, cols] so every
   DMA is 128 large contiguous descriptors.
 - A/B are shipped as UNNORMALIZED numerators + f32 rowsum; the host does
   the division (keeps PSUM-drain work balanced across ACT and DVE).
"""

import numpy as np
import ml_dtypes

import concourse.bass as bass
import concourse.tile as tile
from concourse import bacc, mybir
from concourse.bass_utils import run_bass_kernel_spmd

F32 = mybir.dt.float32
BF16 = mybir.dt.bfloat16
NPBF16 = ml_dtypes.bfloat16
EXP = mybir.ActivationFunctionType.Exp
ALU = mybir.AluOpType

B, LC, LQ, D = 32, 2048, 256, 256
NCORES = 8
BPC = B // NCORES          # batches per core
NKD = D // 128             # 2 contraction chunks over D
NQT = LQ // 128            # 2 query tiles
NEG = np.float32(-1e30)


def _chunks(n, step=512):
    out, off = [], 0
    while off < n:
        w = min(step, n - off)
        out.append((off, w))
        off += w
    return out


def _build_kernel(tc, nc, ins, outs, NC2):
    import contextlib
    NT = NC2 // 128
    FV = 6 + NT + 258          # fvec cols: sqb(2) qf(2) qsc(2) sc(NT) ctxsum(258)
    ctx = contextlib.ExitStack()

    sb = lambda name, bufs: ctx.enter_context(tc.tile_pool(name=name, bufs=bufs))
    psA = ctx.enter_context(tc.tile_pool(name="psA", bufs=4, space="PSUM"))
    psPc = ctx.enter_context(tc.tile_pool(name="psPc", bufs=2, space="PSUM"))
    psT = ctx.enter_context(tc.tile_pool(name="psT", bufs=2, space="PSUM"))

    p_ctxT = sb("pctxT", 2)
    p_ctx = sb("pctx", 2)
    p_qwm = sb("pqwm", 2)
    p_qe = sb("pqe", 2)
    p_fv = sb("pfv", 2)
    p_pt = sb("ppt", 2)
    p_pc = sb("ppc", 2)
    p_tn = sb("ptn", 2)
    p_cs = sb("pcs", 2)
    p_ast = sb("past", 2)
    p_bst = sb("pbst", 2)
    p_rst = sb("prst", 2)

    for b in range(BPC):
        # ---- loads (SBUF-image layouts; one DMA each) ----
        ctxT_sb = p_ctxT.tile([128, NKD, NC2], BF16, name="ctxT_sb")
        nc.sync.dma_start(out=ctxT_sb[:], in_=ins["ctxT2"][b])
        ctx_sb = p_ctx.tile([128, NT, 258], BF16, name="ctx_sb")
        nc.sync.dma_start(out=ctx_sb[:], in_=ins["ctx2"][b])
        qwm_sb = p_qwm.tile([128, NKD, LQ], BF16, name="qwm_sb")
        nc.sync.dma_start(out=qwm_sb[:], in_=ins["qwmT2"][b])
        qe_sb = p_qe.tile([128, NQT, 257], BF16, name="qe_sb")
        nc.sync.dma_start(out=qe_sb[:], in_=ins["qe2"][b])
        fv_sb = p_fv.tile([128, FV], F32, name="fv_sb")
        nc.sync.dma_start(out=fv_sb[:], in_=ins["fvec"][b])
        sqb = lambda qt: fv_sb[:, qt:qt + 1]
        qf = lambda qt: fv_sb[:, 2 + qt:3 + qt]
        qsc = lambda qt: fv_sb[:, 4 + qt:5 + qt]
        scb = lambda ci: fv_sb[:, 6 + ci:7 + ci]
        ctxsum = fv_sb[:, 6 + NT:6 + NT + 258]

        # ---- PT = exp(simT + sq_bias[q]) : [q, c'] orientation ----
        PT_sb = p_pt.tile([128, NQT, NC2], BF16, name="PT_sb")
        for qt in range(NQT):
            for (off, cw) in _chunks(NC2):
                ps = psA.tile([128, 512], F32, tag="psA", name="ps_pt")
                for kd in range(NKD):
                    nc.tensor.matmul(
                        ps[:, 0:cw],
                        lhsT=qwm_sb[:, kd, bass.ts(qt, 128)],
                        rhs=ctxT_sb[:, kd, off:off + cw],
                        start=(kd == 0), stop=(kd == NKD - 1))
                nc.scalar.activation(
                    PT_sb[:, qt, off:off + cw], ps[:, 0:cw], EXP, bias=sqb(qt))

        # ---- Pc = exp(sim + sc_bias[c]) : [c', q]; T accumulation ----
        Pc_sb = p_pc.tile([128, NT, LQ], BF16, name="Pc_sb")
        T_ps = [psT.tile([128, 512], F32, tag="psT", name=f"T_ps{qt}")
                for qt in range(NQT)]

        def emit_pc(ci):
            psc = psPc.tile([128, 512], F32, tag="psPc", name="psc")
            for kd in range(NKD):
                nc.tensor.matmul(
                    psc[:, 0:LQ],
                    lhsT=ctxT_sb[:, kd, bass.ts(ci, 128)],
                    rhs=qwm_sb[:, kd, :],
                    start=(kd == 0), stop=(kd == NKD - 1))
            nc.scalar.activation(Pc_sb[:, ci, :], psc[:, 0:LQ], EXP, bias=scb(ci))

        def emit_t(ci):
            for qt in range(NQT):
                nc.tensor.matmul(
                    T_ps[qt][:, 0:258],
                    lhsT=Pc_sb[:, ci, bass.ts(qt, 128)],
                    rhs=ctx_sb[:, ci, :],
                    start=(ci == 0), stop=(ci == NT - 1))

        for i in range(NT + 2):
            if i < NT:
                emit_pc(i)
            if i >= 2:
                emit_t(i - 2)

        # ---- T finalize: qmask blend + column-softmax normalize ----
        Tn_sb = p_tn.tile([128, NQT, 256], BF16, name="Tn_sb")
        csrec = p_cs.tile([128, NQT], F32, name="csrec")
        for qt in range(NQT):
            nc.vector.tensor_scalar_mul(
                T_ps[qt][:, 0:258], T_ps[qt][:, 0:258], qsc(qt))
            nc.vector.scalar_tensor_tensor(
                out=T_ps[qt][:, 0:258], in0=ctxsum, scalar=qf(qt),
                in1=T_ps[qt][:, 0:258], op0=ALU.mult, op1=ALU.add)
            nc.vector.reciprocal(csrec[:, qt:qt + 1], T_ps[qt][:, 256:257])
            nc.vector.tensor_scalar_mul(
                Tn_sb[:, qt, :], T_ps[qt][:, 0:256], csrec[:, qt:qt + 1])
        nc.sync.dma_start(out=outs["Tno"][b], in_=Tn_sb[:])

        # ---- A_num | B_num (shared stationary PT blocks), rowsum ----
        Ast = p_ast.tile([128, NT, 256], BF16, name="Ast")
        Bst = p_bst.tile([128, NT, 256], BF16, name="Bst")
        Rst = p_rst.tile([128, NT], F32, name="Rst")
        for ci in range(NT):
            pa = psA.tile([128, 512], F32, tag="psA", name="pa")
            for qt in range(NQT):
                nc.tensor.matmul(
                    pa[:, 0:257],
                    lhsT=PT_sb[:, qt, bass.ts(ci, 128)],
                    rhs=qe_sb[:, qt, :],
                    start=(qt == 0), stop=(qt == NQT - 1))
            pb = psA.tile([128, 512], F32, tag="psA", name="pb")
            for qt in range(NQT):
                nc.tensor.matmul(
                    pb[:, 0:256],
                    lhsT=PT_sb[:, qt, bass.ts(ci, 128)],
                    rhs=Tn_sb[:, qt, :],
                    start=(qt == 0), stop=(qt == NQT - 1))
            nc.scalar.copy(Ast[:, ci, :], pa[:, 0:256])
            nc.scalar.copy(Rst[:, ci:ci + 1], pa[:, 256:257])
            nc.vector.tensor_copy(Bst[:, ci, :], pb[:, 0:256])
        nc.sync.dma_start(out=outs["Ao"][b], in_=Ast[:])
        nc.sync.dma_start(out=outs["Bo"][b], in_=Bst[:])
        nc.sync.dma_start(out=outs["Ro"][b], in_=Rst[:])

    ctx.close()


def build_program(NC2):
    NT = NC2 // 128
    FV = 6 + NT + 258
    nc = bacc.Bacc("TRN2", target_bir_lowering=False, debug=False,
                   num_devices=NCORES)
    ins = {
        "ctxT2": nc.dram_tensor("ctxT2", [BPC, 128, NKD, NC2], BF16,
                                kind="ExternalInput").ap(),
        "ctx2": nc.dram_tensor("ctx2", [BPC, 128, NT, 258], BF16,
                               kind="ExternalInput").ap(),
        "qwmT2": nc.dram_tensor("qwmT2", [BPC, 128, NKD, LQ], BF16,
                                kind="ExternalInput").ap(),
        "qe2": nc.dram_tensor("qe2", [BPC, 128, NQT, 257], BF16,
                              kind="ExternalInput").ap(),
        "fvec": nc.dram_tensor("fvec", [BPC, 128, FV], F32,
                               kind="ExternalInput").ap(),
    }
    outs = {
        "Ao": nc.dram_tensor("Ao", [BPC, 128, NT, 256], BF16,
                             kind="ExternalOutput").ap(),
        "Bo": nc.dram_tensor("Bo", [BPC, 128, NT, 256], BF16,
                             kind="ExternalOutput").ap(),
        "Ro": nc.dram_tensor("Ro", [BPC, 128, NT], F32,
                             kind="ExternalOutput").ap(),
        "Tno": nc.dram_tensor("Tno", [BPC, 128, NQT, 256], BF16,
                              kind="ExternalOutput").ap(),
    }
    with tile.TileContext(nc) as tc:
        _build_kernel(tc, nc, ins, outs, NC2)
    nc.compile()
    return nc


def _aux(context_mask):
    """Per-batch unmasked-context indices and the padded compact size."""
    cm = np.asarray(context_mask).astype(bool)
    idx = [np.flatnonzero(~cm[b]) for b in range(cm.shape[0])]
    nmax = max((len(u) for u in idx), default=1)
    NC2 = max(128, ((int(nmax) + 127) // 128) * 128)
    return idx, NC2


def _img(a, p=128):
    """[N*p, X...] row-major -> SBUF image [p, N, X...] (row r = t*p + lane)."""
    n = a.shape[0] // p
    return np.ascontiguousarray(
        a.reshape((n, p) + a.shape[1:]).swapaxes(0, 1))


def host_prep(context, query, context_mask, query_mask, w0):
    """Host-side preprocessing: compact, shard, build device blobs."""
    f = np.float32
    context = np.asarray(context, dtype=f)
    query = np.asarray(query, dtype=f)
    w0 = np.asarray(w0, dtype=f)
    wc, wq, wm = w0[:D], w0[D:2 * D], w0[2 * D:]
    qmf = np.asarray(query_mask).astype(f)                  # (B, LQ)
    idx, NC2 = _aux(context_mask)
    NT = NC2 // 128

    sq = query @ wq                                         # (B, LQ)
    sq_bias = ((1.0 - qmf) * sq + qmf * NEG).astype(f)      # -1e30 on masked q
    qwmT = (query * wm).transpose(0, 2, 1)                  # (B, D, LQ) f32
    ctxsum = context.sum(1, dtype=f)                        # (B, D) over FULL ctx
    qe = np.concatenate([query, np.ones((B, LQ, 1), f)], -1)  # (B, LQ, 257)

    in_maps = []
    for c in range(NCORES):
        m = {"ctxT2": np.zeros((BPC, 128, NKD, NC2), NPBF16),
             "ctx2": np.zeros((BPC, 128, NT, 258), NPBF16),
             "qwmT2": np.empty((BPC, 128, NKD, LQ), NPBF16),
             "qe2": np.empty((BPC, 128, NQT, 257), NPBF16),
             "fvec": np.zeros((BPC, 128, 6 + NT + 258), f)}
        for lb in range(BPC):
            b = c * BPC + lb
            U = idx[b]
            n = len(U)
            cU = context[b][U]                              # (n, D)
            scU = cU @ wc                                   # (n,)
            ctxT_pad = np.zeros((D, NC2), f)
            ctxT_pad[:, :n] = cU.T
            m["ctxT2"][lb] = _img(ctxT_pad).astype(NPBF16)
            ctx_pad = np.zeros((NC2, 258), f)
            ctx_pad[:n, :D] = cU
            ctx_pad[:n, D] = 1.0
            m["ctx2"][lb] = _img(ctx_pad).astype(NPBF16)
            m["qwmT2"][lb] = _img(qwmT[b]).astype(NPBF16)
            m["qe2"][lb] = _img(qe[b]).astype(NPBF16)
            sc_pad = np.full(NC2, NEG, f)
            sc_pad[:n] = scU
            fv = m["fvec"][lb]
            fv[:, 0:2] = sq_bias[b].reshape(NQT, 128).T
            fv[:, 2:4] = qmf[b].reshape(NQT, 128).T
            fv[:, 4:6] = (1.0 - qmf[b]).reshape(NQT, 128).T
            fv[:, 6:6 + NT] = sc_pad.reshape(NT, 128).T
            fv[:, 6 + NT:6 + NT + D] = ctxsum[b][None, :]
            fv[:, 6 + NT + D] = float(LC)
        in_maps.append(m)
    return in_maps


_cached_nc = {}


def get_program(NC2):
    if NC2 not in _cached_nc:
        _cached_nc[NC2] = build_program(NC2)
    return _cached_nc[NC2]


def run_on_hw(in_maps, **kwargs):
    NC2 = in_maps[0]["ctxT2"].shape[-1]
    nc = get_program(NC2)
    return run_bass_kernel_spmd(nc, in_maps, core_ids=list(range(NCORES)),
                                **kwargs)


def kernel(context, query, context_mask, query_mask, w0):
    f = np.float32
    context = np.asarray(context, dtype=f)
    query = np.asarray(query, dtype=f)
    idx, NC2 = _aux(context_mask)
    NT = NC2 // 128
    in_maps = host_prep(context, query, context_mask, query_mask, w0)
    res = run_on_hw(in_maps)

    A = np.empty((B, LC, D), f)
    Bm = np.empty((B, LC, D), f)
    cmask = np.asarray(context_mask).astype(bool)
    for c in range(NCORES):
        r = res.results[c]
        for lb in range(BPC):
            b = c * BPC + lb
            U = idx[b]
            n = len(U)
            Araw = r["Ao"][lb].astype(f).swapaxes(0, 1).reshape(NC2, D)
            Braw = r["Bo"][lb].astype(f).swapaxes(0, 1).reshape(NC2, D)
            rs = r["Ro"][lb].astype(f).swapaxes(0, 1).reshape(NC2)
            Tn = r["Tno"][lb].astype(f).swapaxes(0, 1).reshape(LQ, D)
            inv = 1.0 / rs[:n, None]
            A[b][U] = Araw[:n] * inv
            Bm[b][U] = Braw[:n] * inv
            mrow = cmask[b]
            A[b][mrow] = query[b].mean(0, dtype=np.float64).astype(f)
            Bm[b][mrow] = Tn.mean(0, dtype=np.float64).astype(f)
    return A, Bm


# revision 9
# speedup vs baseline: 2.6132x; 1.1041x over previous
"""Trainium2 Bass kernel for ContextQueryAttn (BiDAF-style trilinear attention).

Computes, per batch b:
    sim = sc[:,None] + sq[None,:] + (ctx*wm) @ query.T          (Lc, Lq)
    sim = where(cmask[:,None] | qmask[None,:], -1e30, sim)
    S   = softmax(sim, axis=-1)   (row softmax over Lq)
    SS  = softmax(sim, axis=0)    (col softmax over Lc)
    A   = S @ query               (Lc, D)
    T   = SS.T @ ctx              (Lq, D)
    B   = S @ T                   (Lc, D)
returns (A, B).

v3 strategy (on top of v2's bf16 + context-mask compaction):
 - per-q factors cancel in the column softmax, so Pc = exp(cross) with NO
   bias; the e^{sc[c]} weight is folded into the ctx rows on the host.
   This merges Pc EXPs into 512-col instructions.
 - rowsum ships as a bf16 column of Ao (no separate f32 Ro copies).
 - PSUM->SBUF drains spread across ACT, DVE and GpSimd.
 - qmask blend operand qf*ctxsum precomputed on host (one DVE op saved).
 - input/output DMAs split into chunks for startup/tail overlap.
"""

import numpy as np
import ml_dtypes

import concourse.bass as bass
import concourse.tile as tile
from concourse import bacc, mybir
from concourse.bass_utils import run_bass_kernel_spmd

F32 = mybir.dt.float32
BF16 = mybir.dt.bfloat16
NPBF16 = ml_dtypes.bfloat16
EXP = mybir.ActivationFunctionType.Exp
ALU = mybir.AluOpType

B, LC, LQ, D = 32, 2048, 256, 256
NCORES = 8
BPC = B // NCORES          # batches per core
NKD = D // 128             # 2 contraction chunks over D
NQT = LQ // 128            # 2 query tiles
NEG = np.float32(-1e30)


def _chunks(n, step=512):
    out, off = [], 0
    while off < n:
        w = min(step, n - off)
        out.append((off, w))
        off += w
    return out


def _build_kernel(tc, nc, ins, outs, NC2):
    import contextlib
    NT = NC2 // 128
    FV = 4 + 2 * 258           # fvec cols: sqb(2) qsc(2) qfc(2*258)
    ctx = contextlib.ExitStack()

    sb = lambda name, bufs: ctx.enter_context(tc.tile_pool(name=name, bufs=bufs))
    # psA tiles are 2 banks (1024 f32) each: PT chunk pairs / A|B pairs.
    psA = ctx.enter_context(tc.tile_pool(name="psA", bufs=2, space="PSUM"))
    psPc = ctx.enter_context(tc.tile_pool(name="psPc", bufs=2, space="PSUM"))
    psT = ctx.enter_context(tc.tile_pool(name="psT", bufs=2, space="PSUM"))

    p_ctxT = sb("pctxT", 2)
    p_ctx = sb("pctx", 2)
    p_qwm = sb("pqwm", 2)
    p_qe = sb("pqe", 2)
    p_fv = sb("pfv", 2)
    p_pt = sb("ppt", 2)
    p_pc = sb("ppc", 2)
    p_tn = sb("ptn", 2)
    p_cs = sb("pcs", 2)
    p_ast = sb("past", 2)

    # Pc tile pairs for merged 512-col EXPs: (ci, ci+1) or a trailing solo.
    ci_pairs = []
    i = 0
    while i < NT:
        ci_pairs.append((i, min(2, NT - i)))
        i += 2

    for b in range(BPC):
        # ---- loads (SBUF-image layouts; split for queue parallelism) ----
        ctxT_sb = p_ctxT.tile([128, NKD, NC2], BF16, name="ctxT_sb")
        for kd in range(NKD):
            nc.sync.dma_start(out=ctxT_sb[:, kd, :], in_=ins["ctxT2"][b, :, kd])
        qwm_sb = p_qwm.tile([128, NKD, LQ], BF16, name="qwm_sb")
        nc.sync.dma_start(out=qwm_sb[:], in_=ins["qwmT2"][b])
        fv_sb = p_fv.tile([128, FV], F32, name="fv_sb")
        nc.sync.dma_start(out=fv_sb[:], in_=ins["fvec"][b])
        ctx_sb = p_ctx.tile([128, NT, 258], BF16, name="ctx_sb")
        h = (NT + 1) // 2
        nc.sync.dma_start(out=ctx_sb[:, 0:h, :], in_=ins["ctx2"][b, :, 0:h])
        nc.sync.dma_start(out=ctx_sb[:, h:NT, :], in_=ins["ctx2"][b, :, h:NT])
        qe_sb = p_qe.tile([128, NQT, 257], BF16, name="qe_sb")
        nc.sync.dma_start(out=qe_sb[:], in_=ins["qe2"][b])
        sqb = lambda qt: fv_sb[:, qt:qt + 1]
        qsc = lambda qt: fv_sb[:, 2 + qt:3 + qt]
        qfc = lambda qt: fv_sb[:, 4 + 258 * qt:4 + 258 * (qt + 1)]

        # ---- PT = exp(simT + sq_bias[q]) : [q, c'] orientation ----
        # 512-col MM chunks pair up in 2-bank psum tiles => merged EXPs.
        PT_sb = p_pt.tile([128, NQT, NC2], BF16, name="PT_sb")
        pt_groups = []
        chl = _chunks(NC2)
        i = 0
        while i < len(chl):
            if i + 1 < len(chl) and chl[i][1] == 512 and chl[i + 1][1] == 512:
                pt_groups.append((chl[i][0], 1024))
                i += 2
            else:
                pt_groups.append(chl[i])
                i += 1
        for qt in range(NQT):
            for (off, gw) in pt_groups:
                ps = psA.tile([128, 1024], F32, tag="psA", name="ps_pt")
                for (o2, cw) in _chunks(gw):
                    for kd in range(NKD):
                        nc.tensor.matmul(
                            ps[:, o2:o2 + cw],
                            lhsT=qwm_sb[:, kd, bass.ts(qt, 128)],
                            rhs=ctxT_sb[:, kd, off + o2:off + o2 + cw],
                            start=(kd == 0), stop=(kd == NKD - 1))
                nc.scalar.activation(
                    PT_sb[:, qt, off:off + gw], ps[:, 0:gw], EXP, bias=sqb(qt))

        # ---- Pc = exp(cross) [c', q] (e^{sc} folded into ctx rows);
        #      T accumulation over c' ----
        Pc_sb = p_pc.tile([128, NT * LQ], BF16, name="Pc_sb")
        T_ps = [psT.tile([128, 512], F32, tag="psT", name=f"T_ps{qt}")
                for qt in range(NQT)]

        def emit_pc(pi):
            ci0, w = ci_pairs[pi]
            psc = psPc.tile([128, 512], F32, tag="psPc", name="psc")
            for j in range(w):
                for kd in range(NKD):
                    nc.tensor.matmul(
                        psc[:, j * LQ:(j + 1) * LQ],
                        lhsT=ctxT_sb[:, kd, bass.ts(ci0 + j, 128)],
                        rhs=qwm_sb[:, kd, :],
                        start=(kd == 0), stop=(kd == NKD - 1))
            nc.scalar.activation(
                Pc_sb[:, ci0 * LQ:(ci0 + w) * LQ], psc[:, 0:w * LQ], EXP)

        def emit_t(ci):
            for qt in range(NQT):
                nc.tensor.matmul(
                    T_ps[qt][:, 0:258],
                    lhsT=Pc_sb[:, ci * LQ + qt * 128:ci * LQ + qt * 128 + 128],
                    rhs=ctx_sb[:, ci, :],
                    start=(ci == 0), stop=(ci == NT - 1))

        # software-pipeline: T MMs for pair pi-1 run while pair pi EXPs
        for pi in range(len(ci_pairs) + 1):
            if pi < len(ci_pairs):
                emit_pc(pi)
            if pi >= 1:
                ci0, w = ci_pairs[pi - 1]
                for j in range(w):
                    emit_t(ci0 + j)

        # ---- T finalize: qmask blend + column-softmax normalize ----
        Tn_sb = p_tn.tile([128, NQT, 256], BF16, name="Tn_sb")
        csrec = p_cs.tile([128, NQT], F32, name="csrec")
        for qt in range(NQT):
            nc.vector.scalar_tensor_tensor(
                out=T_ps[qt][:, 0:258], in0=T_ps[qt][:, 0:258], scalar=qsc(qt),
                in1=qfc(qt), op0=ALU.mult, op1=ALU.add)
            nc.vector.reciprocal(csrec[:, qt:qt + 1], T_ps[qt][:, 256:257])
            nc.vector.tensor_scalar_mul(
                Tn_sb[:, qt, :], T_ps[qt][:, 0:256], csrec[:, qt:qt + 1])
        nc.sync.dma_start(out=outs["Tno"][b], in_=Tn_sb[:])

        # ---- A_num | B_num (shared stationary PT blocks) in one 2-bank
        #      psum tile: A+rowsum at cols 0:257, B at 512:768; drained by
        #      ONE strided copy per ci -> ABst row [A(256),rsum,B(256),junk]
        ABst = p_ast.tile([128, NT * 514], BF16, name="ABst")
        ab_flush = 0
        for ci in range(NT):
            pab = psA.tile([128, 1024], F32, tag="psA", name="pab")
            for qt in range(NQT):
                nc.tensor.matmul(
                    pab[:, 0:257],
                    lhsT=PT_sb[:, qt, bass.ts(ci, 128)],
                    rhs=qe_sb[:, qt, :],
                    start=(qt == 0), stop=(qt == NQT - 1))
            for qt in range(NQT):
                nc.tensor.matmul(
                    pab[:, 512:768],
                    lhsT=PT_sb[:, qt, bass.ts(ci, 128)],
                    rhs=Tn_sb[:, qt, :],
                    start=(qt == 0), stop=(qt == NQT - 1))
            src = pab[:].rearrange("p (g x) -> p g x", g=2)[:, :, 0:257]
            dst = ABst[:, ci * 514:(ci + 1) * 514].rearrange(
                "p (g x) -> p g x", g=2)[:, :, 0:257]
            if ci % 3 == 2:
                nc.scalar.copy(dst, src)
            else:
                nc.vector.tensor_copy(dst, src)
            # flush output chunks early so the DMA tail overlaps compute
            if ci == NT // 3 or ci == (2 * NT) // 3 or ci == NT - 1:
                nc.sync.dma_start(
                    out=outs["ABo"][b, :, ab_flush * 514:(ci + 1) * 514],
                    in_=ABst[:, ab_flush * 514:(ci + 1) * 514])
                ab_flush = ci + 1

    ctx.close()


def build_program(NC2):
    NT = NC2 // 128
    FV = 4 + 2 * 258
    nc = bacc.Bacc("TRN2", target_bir_lowering=False, debug=False,
                   num_devices=NCORES)
    ins = {
        "ctxT2": nc.dram_tensor("ctxT2", [BPC, 128, NKD, NC2], BF16,
                                kind="ExternalInput").ap(),
        "ctx2": nc.dram_tensor("ctx2", [BPC, 128, NT, 258], BF16,
                               kind="ExternalInput").ap(),
        "qwmT2": nc.dram_tensor("qwmT2", [BPC, 128, NKD, LQ], BF16,
                                kind="ExternalInput").ap(),
        "qe2": nc.dram_tensor("qe2", [BPC, 128, NQT, 257], BF16,
                              kind="ExternalInput").ap(),
        "fvec": nc.dram_tensor("fvec", [BPC, 128, FV], F32,
                               kind="ExternalInput").ap(),
    }
    outs = {
        "ABo": nc.dram_tensor("ABo", [BPC, 128, NT * 514], BF16,
                              kind="ExternalOutput").ap(),
        "Tno": nc.dram_tensor("Tno", [BPC, 128, NQT, 256], BF16,
                              kind="ExternalOutput").ap(),
    }
    with tile.TileContext(nc) as tc:
        _build_kernel(tc, nc, ins, outs, NC2)
    nc.compile()
    return nc


def _aux(context_mask):
    """Per-batch unmasked-context indices and the padded compact size."""
    cm = np.asarray(context_mask).astype(bool)
    idx = [np.flatnonzero(~cm[b]) for b in range(cm.shape[0])]
    nmax = max((len(u) for u in idx), default=1)
    NC2 = max(128, ((int(nmax) + 127) // 128) * 128)
    return idx, NC2


def _img(a, p=128):
    """[N*p, X...] row-major -> SBUF image [p, N, X...] (row r = t*p + lane)."""
    n = a.shape[0] // p
    return np.ascontiguousarray(
        a.reshape((n, p) + a.shape[1:]).swapaxes(0, 1))


def host_prep(context, query, context_mask, query_mask, w0):
    """Host-side preprocessing: compact, shard, build device blobs."""
    f = np.float32
    context = np.asarray(context, dtype=f)
    query = np.asarray(query, dtype=f)
    w0 = np.asarray(w0, dtype=f)
    wc, wq, wm = w0[:D], w0[D:2 * D], w0[2 * D:]
    qmf = np.asarray(query_mask).astype(f)                  # (B, LQ)
    idx, NC2 = _aux(context_mask)
    NT = NC2 // 128

    sq = query @ wq                                         # (B, LQ)
    sq_bias = ((1.0 - qmf) * sq + qmf * NEG).astype(f)      # -1e30 on masked q
    qwmT = (query * wm).transpose(0, 2, 1)                  # (B, D, LQ) f32
    ctxsum = context.sum(1, dtype=f)                        # (B, D) over FULL ctx
    ctxsum_ext = np.concatenate(
        [ctxsum, np.full((B, 1), LC, f), np.zeros((B, 1), f)], -1)  # (B, 258)
    qe = np.concatenate([query, np.ones((B, LQ, 1), f)], -1)  # (B, LQ, 257)

    in_maps = []
    for c in range(NCORES):
        m = {"ctxT2": np.zeros((BPC, 128, NKD, NC2), NPBF16),
             "ctx2": np.zeros((BPC, 128, NT, 258), NPBF16),
             "qwmT2": np.empty((BPC, 128, NKD, LQ), NPBF16),
             "qe2": np.empty((BPC, 128, NQT, 257), NPBF16),
             "fvec": np.zeros((BPC, 128, 4 + 2 * 258), f)}
        for lb in range(BPC):
            b = c * BPC + lb
            U = idx[b]
            n = len(U)
            cU = context[b][U]                              # (n, D)
            scU = cU @ wc                                   # (n,)
            ctxT_pad = np.zeros((D, NC2), f)
            ctxT_pad[:, :n] = cU.T
            m["ctxT2"][lb] = _img(ctxT_pad).astype(NPBF16)
            # ctx rows scaled by e^{sc[c]} (column-softmax weight); the
            # ones-col picks up the same factor => correct normalizer.
            ctx_pad = np.zeros((NC2, 258), f)
            ctx_pad[:n, :D] = cU
            ctx_pad[:n, D] = 1.0
            ctx_pad[:n] *= np.exp(scU, dtype=f)[:, None]
            m["ctx2"][lb] = _img(ctx_pad).astype(NPBF16)
            m["qwmT2"][lb] = _img(qwmT[b]).astype(NPBF16)
            m["qe2"][lb] = _img(qe[b]).astype(NPBF16)
            fv = m["fvec"][lb]
            fv[:, 0:2] = sq_bias[b].reshape(NQT, 128).T
            fv[:, 2:4] = (1.0 - qmf[b]).reshape(NQT, 128).T
            # qfc[qt] = qf[q] * ctxsum_ext  (rank-1, per q-tile)
            for qt in range(NQT):
                fv[:, 4 + 258 * qt:4 + 258 * (qt + 1)] = (
                    qmf[b][qt * 128:(qt + 1) * 128, None] * ctxsum_ext[b][None, :])
        in_maps.append(m)
    return in_maps


_cached_nc = {}


def get_program(NC2):
    if NC2 not in _cached_nc:
        _cached_nc[NC2] = build_program(NC2)
    return _cached_nc[NC2]


def run_on_hw(in_maps, **kwargs):
    NC2 = in_maps[0]["ctxT2"].shape[-1]
    nc = get_program(NC2)
    return run_bass_kernel_spmd(nc, in_maps, core_ids=list(range(NCORES)),
                                **kwargs)


def kernel(context, query, context_mask, query_mask, w0):
    f = np.float32
    context = np.asarray(context, dtype=f)
    query = np.asarray(query, dtype=f)
    idx, NC2 = _aux(context_mask)
    NT = NC2 // 128
    in_maps = host_prep(context, query, context_mask, query_mask, w0)
    res = run_on_hw(in_maps)

    A = np.empty((B, LC, D), f)
    Bm = np.empty((B, LC, D), f)
    cmask = np.asarray(context_mask).astype(bool)
    for c in range(NCORES):
        r = res.results[c]
        for lb in range(BPC):
            b = c * BPC + lb
            U = idx[b]
            n = len(U)
            ABr = r["ABo"][lb].astype(f).reshape(128, NT, 514).swapaxes(0, 1)
            ABr = ABr.reshape(NC2, 514)
            Tn = r["Tno"][lb].astype(f).swapaxes(0, 1).reshape(LQ, D)
            inv = 1.0 / ABr[:n, 256:257]
            A[b][U] = ABr[:n, 0:256] * inv
            Bm[b][U] = ABr[:n, 257:513] * inv
            mrow = cmask[b]
            A[b][mrow] = query[b].mean(0, dtype=np.float64).astype(f)
            Bm[b][mrow] = Tn.mean(0, dtype=np.float64).astype(f)
    return A, Bm
